# revision 1
# baseline (speedup 1.0000x reference)
"""MultiHeadAttention Trainium2 kernel (8 NeuronCores, SPMD).

Problem: B=2, L=2048, DK=DV=512, H=8, dh=64.
  Q = q @ WQ[h]; K = k @ WK[h]; V = v @ WV[h]       (per head)
  y = Q K^T / sqrt(L); z = softmax(y, axis=QUERY); out = z @ V
  concat heads on feature dim.

Sharding: 16 (b,h) pairs over 8 cores -> 2 heads (same batch) per core.
Host marshals per-core inputs: activations transposed to [D, L] and cast to
bf16, the core's two heads' weights packed to [D, 128].

Device layout (per core, heads h0/h1 packed in partition halves):
  QT/KT = [e-pack(128), L]   (rows 0:64 head0's Q^T, 64:128 head1's)
  S'[k-tile, q] scores with k on partitions, q on free axis -> softmax over q
  is a free-axis row-sum: exp on ScalarE with fused accum_out row sums.
  1/D is folded into V rows (k-indexed), then out^T[ev,q] accumulates in PSUM.
Core output: [128, 2048] f32 = two heads' out^T stacked; host transposes back.
"""

import math

import numpy as np

B = 2
L = 2048
DK = 512
H = 8
DH = 64
P = 128
NKT = L // P  # 16 k-tiles
NQC = L // 512  # 4 q-chunks
NDC = DK // P  # 4 d-chunks
N_CORES = 8

_CACHE = {}


def _build_program():
    import concourse.bass as bass
    import concourse.tile as tile
    from concourse import bacc, mybir
    from concourse.bass import ts

    f32 = mybir.dt.float32
    bf16 = mybir.dt.bfloat16
    AF = mybir.ActivationFunctionType
    SCALE = 1.0 / math.sqrt(float(L))

    nc = bacc.Bacc("TRN2", target_bir_lowering=False, debug=False)

    qt_d = nc.dram_tensor("qt", [DK, L], bf16, kind="ExternalInput")
    kt_d = nc.dram_tensor("kt", [DK, L], bf16, kind="ExternalInput")
    vt_d = nc.dram_tensor("vt", [DK, L], bf16, kind="ExternalInput")
    wq_d = nc.dram_tensor("wq", [DK, P], bf16, kind="ExternalInput")
    wk_d = nc.dram_tensor("wk", [DK, P], bf16, kind="ExternalInput")
    wv_d = nc.dram_tensor("wv", [DK, P], bf16, kind="ExternalInput")
    out_d = nc.dram_tensor("out", [P, L], f32, kind="ExternalOutput")

    with tile.TileContext(nc) as tc:
        with (
            tc.tile_pool(name="consts", bufs=1) as consts,
            tc.tile_pool(name="xin", bufs=1) as xin,
            tc.tile_pool(name="proj", bufs=1) as proj,
            tc.tile_pool(name="epool", bufs=4) as epool,
            tc.tile_pool(name="vspool", bufs=4) as vspool,
            tc.tile_pool(name="stats", bufs=1) as stats,
            tc.tile_pool(name="outp", bufs=4) as outp,
            tc.tile_pool(name="spsum", bufs=2, space="PSUM") as spsum,
            tc.tile_pool(name="avpsum", bufs=1, space="PSUM") as avpsum,
        ):
            # ---- chunked loads, ordered so the first exp's dependencies
            # (wq, wk, qt01, kt0) land first ----
            wq_s = consts.tile([P, NDC, P], bf16)
            wk_s = consts.tile([P, NDC, P], bf16)
            wv_s = consts.tile([P, NDC, P], bf16)
            qt_s = xin.tile([P, NDC, L], bf16)
            kt_s = xin.tile([P, NDC, L], bf16)
            vt_s = xin.tile([P, NDC, L], bf16)
            qt_r = qt_d.rearrange("(o p) l -> p o l", p=P)
            kt_r = kt_d.rearrange("(o p) l -> p o l", p=P)
            vt_r = vt_d.rearrange("(o p) l -> p o l", p=P)

            def load_chunk(sb, rr, c):
                nc.sync.dma_start(sb[:, :, ts(c, 512)], rr[:, :, ts(c, 512)])

            nc.sync.dma_start(wq_s[:], wq_d.rearrange("(o p) e -> p o e", p=P))
            load_chunk(qt_s, qt_r, 0)
            nc.sync.dma_start(wk_s[:], wk_d.rearrange("(o p) e -> p o e", p=P))
            load_chunk(qt_s, qt_r, 1)
            nc.sync.dma_start(kt_s[:, :, 0:128], kt_r[:, :, 0:128])
            nc.sync.dma_start(kt_s[:, :, 128:512], kt_r[:, :, 128:512])
            load_chunk(qt_s, qt_r, 2)
            load_chunk(qt_s, qt_r, 3)
            nc.sync.dma_start(wv_s[:], wv_d.rearrange("(o p) e -> p o e", p=P))
            load_chunk(vt_s, vt_r, 0)
            load_chunk(kt_s, kt_r, 1)
            load_chunk(vt_s, vt_r, 1)
            load_chunk(kt_s, kt_r, 2)
            load_chunk(vt_s, vt_r, 2)
            load_chunk(kt_s, kt_r, 3)
            load_chunk(vt_s, vt_r, 3)

            QT = proj.tile([P, L], bf16)
            KT = proj.tile([P, L], bf16)
            Vf = proj.tile([P, NKT, P], f32)

            Dsum = stats.tile([P, 2, NKT, 2], f32)
            Dsum0 = stats.tile([P, 2, NQC], f32)
            Dtot = stats.tile([P, 2, NKT], f32)
            Drec = stats.tile([P, 2, NKT], f32)

            # AV accumulators (one bank per q-chunk; separate tensors so
            # tail evacuation copies carry no cross-chunk deps):
            # rows 0:64 head0 out^T, 64:128 head1 out^T
            ovs = [
                avpsum.tile([P, 512], f32, name=f"ovs{qc}") for qc in range(NQC)
            ]

            def k_proj_chunk(c):
                ps = spsum.tile([P, 1024], f32, tag="sco", name="kproj")
                for dc in range(NDC):
                    nc.tensor.matmul(
                        ps[:, 0:512],
                        lhsT=wk_s[:, dc, :],
                        rhs=kt_s[:, dc, ts(c, 512)],
                        start=(dc == 0),
                        stop=(dc == NDC - 1),
                    )
                nc.vector.tensor_copy(KT[:, ts(c, 512)], ps[:, 0:512])

            def q_proj_chunk(qc):
                ps = spsum.tile([P, 1024], f32, tag="sco", name="qproj")
                for dc in range(NDC):
                    nc.tensor.matmul(
                        ps[:, 0:512],
                        lhsT=wq_s[:, dc, :],
                        rhs=qt_s[:, dc, ts(qc, 512)],
                        start=(dc == 0),
                        stop=(dc == NDC - 1),
                    )
                nc.vector.tensor_copy(QT[:, ts(qc, 512)], ps[:, 0:512])

            # Only what the first scores burst needs: QT half 0 + KT chunk 0
            q_proj_chunk(0)
            q_proj_chunk(1)
            psk = spsum.tile([P, 1024], f32, tag="sco", name="kproj0a")
            for dc in range(NDC):
                nc.tensor.matmul(
                    psk[:, 0:128],
                    lhsT=wk_s[:, dc, :],
                    rhs=kt_s[:, dc, 0:128],
                    start=(dc == 0),
                    stop=(dc == NDC - 1),
                )
            nc.vector.tensor_copy(KT[:, 0:128], psk[:, 0:128])

            # ---- main loop over k-tiles ----
            for kt in range(NKT):
                def scores_exp(h, half):
                    hp = h * DH  # partition offset of this head's rows
                    ps = spsum.tile([P, 1024], f32, tag="sco", name="sco")
                    for j in range(2):
                        qc = half * 2 + j
                        nc.tensor.matmul(
                            ps[:, ts(j, 512)],
                            lhsT=KT[hp : hp + DH, ts(kt, P)],
                            rhs=QT[hp : hp + DH, ts(qc, 512)],
                            start=True,
                            stop=True,
                        )
                    nc.scalar.activation(
                        Etiles[h][:, ts(half, 1024)],
                        ps[:],
                        AF.Exp,
                        scale=SCALE,
                        accum_out=Dsum[:, h : h + 1, kt : kt + 1, half : half + 1],
                    )


                Etiles = []
                for h in range(2):
                    Etiles.append(epool.tile([P, L], bf16, tag="E", name=f"E{h}"))
                # slot order keeps ACT fed: projections slip into the slots
                # freed between score rounds
                def v_proj(k_chunk=None):
                    # V projection for this k-tile: [k(128), ev-pack(128)];
                    # a due K-proj chunk shares the slot (columns 512:1024)
                    psv = spsum.tile([P, 1024], f32, tag="sco", name="psv")
                    for dc in range(NDC):
                        nc.tensor.matmul(
                            psv[:, 0:P],
                            lhsT=vt_s[:, dc, ts(kt, P)],
                            rhs=wv_s[:, dc, :],
                            start=(dc == 0),
                            stop=(dc == NDC - 1),
                        )
                    nc.vector.tensor_copy(Vf[:, kt, :], psv[:, 0:P])
                    if k_chunk == "rest0":
                        for dc in range(NDC):
                            nc.tensor.matmul(
                                psv[:, 512:896],
                                lhsT=wk_s[:, dc, :],
                                rhs=kt_s[:, dc, 128:512],
                                start=(dc == 0),
                                stop=(dc == NDC - 1),
                            )
                        nc.vector.tensor_copy(KT[:, 128:512], psv[:, 512:896])
                    elif k_chunk is not None:
                        for dc in range(NDC):
                            nc.tensor.matmul(
                                psv[:, 512:1024],
                                lhsT=wk_s[:, dc, :],
                                rhs=kt_s[:, dc, ts(k_chunk, 512)],
                                start=(dc == 0),
                                stop=(dc == NDC - 1),
                            )
                        nc.vector.tensor_copy(
                            KT[:, ts(k_chunk, 512)], psv[:, 512:1024]
                        )

                scores_exp(0, 0)
                scores_exp(1, 0)
                if kt == 0:
                    q_proj_chunk(2)
                    q_proj_chunk(3)
                if kt == 0:
                    kc = "rest0"
                elif kt % 4 == 1 and kt < 12:
                    kc = kt // 4 + 1
                else:
                    kc = None
                v_proj(k_chunk=kc)
                scores_exp(0, 1)
                scores_exp(1, 1)
                for h in range(2):
                    hp = h * DH
                    E = Etiles[h]
                    # D for these 128 k-rows is complete
                    nc.vector.tensor_add(
                        Dtot[:, h : h + 1, kt : kt + 1],
                        Dsum[:, h : h + 1, kt : kt + 1, 0:1],
                        Dsum[:, h : h + 1, kt : kt + 1, 1:2],
                    )
                    nc.vector.reciprocal(
                        Drec[:, h : h + 1, kt : kt + 1],
                        Dtot[:, h : h + 1, kt : kt + 1],
                    )
                    # V~ = V / D, zero-padded into the other head's half
                    Vs = vspool.tile([P, P], bf16, tag="vs")
                    nc.gpsimd.memset(Vs[:, ts(1 - h, DH)], 0.0)
                    nc.vector.tensor_scalar_mul(
                        Vs[:, ts(h, DH)],
                        Vf[:, kt, ts(h, DH)],
                        Drec[:, h : h + 1, kt : kt + 1],
                    )
                    # out^T accumulation: rows hp:hp+64 get this head's result
                    for qc in range(NQC):
                        nc.tensor.matmul(
                            ovs[qc][:],
                            lhsT=Vs[:],
                            rhs=E[:, ts(qc, 512)],
                            start=(kt == 0 and h == 0),
                            stop=(kt == NKT - 1 and h == 1),
                            skip_group_check=True,
                        )

            # tail: all accumulation closed; evacuate + store per q-chunk,
            # copies split across DVE and the now-idle ACT
            for qc in range(NQC):
                oc = outp.tile([P, 512], f32, tag="oc", bufs=4)
                if qc % 2 == 0:
                    nc.vector.tensor_copy(oc[:], ovs[qc][:])
                else:
                    nc.scalar.copy(oc[:], ovs[qc][:])
                nc.sync.dma_start(out_d[:, ts(qc, 512)], oc[:])

    nc.compile()
    return nc


def _get_program():
    if "nc" not in _CACHE:
        _CACHE["nc"] = _build_program()
    return _CACHE["nc"]


def kernel(keys, queries, values, WQ, WK, WV):
    import ml_dtypes

    from concourse import bass_utils

    bf = ml_dtypes.bfloat16
    keys = np.asarray(keys)
    queries = np.asarray(queries)
    values = np.asarray(values)
    WQ = np.asarray(WQ)
    WK = np.asarray(WK)
    WV = np.asarray(WV)

    nc = _get_program()

    in_maps = []
    for c in range(N_CORES):
        b = c // 4
        h0 = 2 * (c % 4)
        h1 = h0 + 1
        in_maps.append(
            {
                "qt": np.ascontiguousarray(queries[b].T).astype(bf),
                "kt": np.ascontiguousarray(keys[b].T).astype(bf),
                "vt": np.ascontiguousarray(values[b].T).astype(bf),
                "wq": np.concatenate([WQ[h0], WQ[h1]], axis=1).astype(bf),
                "wk": np.concatenate([WK[h0], WK[h1]], axis=1).astype(bf),
                "wv": np.concatenate([WV[h0], WV[h1]], axis=1).astype(bf),
            }
        )

    res = bass_utils.run_bass_kernel_spmd(nc, in_maps, core_ids=list(range(N_CORES)))

    out = np.empty((B, L, H * DH), dtype=np.float32)
    for c in range(N_CORES):
        b = c // 4
        h0 = 2 * (c % 4)
        ot = res.results[c]["out"]  # [128, L]
        out[b, :, h0 * DH : (h0 + 1) * DH] = ot[0:DH, :].T
        out[b, :, (h0 + 1) * DH : (h0 + 2) * DH] = ot[DH : 2 * DH, :].T
    return out

